# revision 16
# baseline (speedup 1.0000x reference)
"""Trainium2 Bass kernel for nn_MultiHeadCrossAttention (v2).

Problem: B=8, C=512, H=W=32 (S=1024), 8 heads x d=64.
Sharding: data-parallel, one batch element per NeuronCore, no collectives.

v2 design (all-fp16 operands, fp32 PSUM accumulation):
  - Bias algebra: softmax(  (q+bq).(k+bk) ) == softmax( (q+bq).k ) since the
    bk cross-term depends only on the query pixel (cancels in the softmax
    normalization) -- bk is dropped entirely; bq is folded into the Q
    projection eviction; bv is re-added at finalize from a host-replicated
    [128, 512] table.
  - Projections q,k in [c, s] layout (P tiles, fp16, k without bias); v is
    projected directly in transposed [t, c] layout (V' tiles per head carry
    a ones column so the AV matmul emits softmax row sums for free).
  - QK computed transposed (scoresT[t, s]) into [128, 1024] PSUM; exp on ACT
    (scale=0.125, no bias) -> fp16 expt tiles.
  - AV *reoriented*: out[s-chunk, 65] accumulates over t-chunks with
    lhsT = expt[:, sc-cols] (stationary, free ldweights) and rhs = V'_head
    [128, 65] (moving).  Output lands directly in [s, head-col] layout:
    no PE transposes, no O evictions.  Two [128, 512] PSUM quads per head
    hold the 8 s-chunks at 128-col offsets.
  - Finalize per head: DVE reciprocal of the rowsum columns + per-quad
    broadcast multiply into the fp32 assembly tile; GPSIMD adds bv in place
    (SBUF-only, keeps DVE/ACT free).  Output DMA per head pair.
  - Schedule: PE warm-up matmuls at t~0; DMA ordered q-side -> k-side with
    chunked transfers so the first exp fires as early as possible; the
    j1..j3 / v projections are sprinkled into the head 0-1 attention stream;
    AV(h) rides head h+1 (AV(7) rides head 7 with lag 1 + a small tail).
  - ACT (exp: 64 x [128,1024] tiles) is the modeled bottleneck engine.
"""

import numpy as np

import concourse.bass as bass  # noqa: F401
import concourse.mybir as mybir
import concourse.tile as tile
from concourse import bacc, bass_utils

F32 = mybir.dt.float32
F16 = mybir.dt.float16

C = 512
S = 1024
NH = 8
D = 64
NKC = C // 128      # 4 contraction chunks
NJ = C // 128       # 4 output chunks (head pairs)
TCHUNK = S // 128   # 8 t-chunks / s-chunks
N_CORES = 8
N_WARM = 6          # PE warm-up matmuls

_CACHE = {}


def _build():
    nc = bacc.Bacc()

    # chunks 0..3: xq kc-chunks; 4..7: xk kc-chunks
    xqk = nc.dram_tensor("xqk", [128, 2 * NKC, S], F16, kind="ExternalInput")
    xv = nc.dram_tensor("xv", [128, NKC, S], F16, kind="ExternalInput")
    # [j][p = c_in % 128][kc][m = c_out - j*128]
    wq = nc.dram_tensor("wq", [NJ, 128, NKC, 128], F16, kind="ExternalInput")
    wk = nc.dram_tensor("wk", [NJ, 128, NKC, 128], F16, kind="ExternalInput")
    # [p = c_in % 128][kc][c_out]
    wv = nc.dram_tensor("wv", [128, NKC, C], F16, kind="ExternalInput")
    bqd = nc.dram_tensor("bqd", [128, NJ], F32, kind="ExternalInput")
    bvb = nc.dram_tensor("bvb", [128, C], F32, kind="ExternalInput")
    out = nc.dram_tensor("out", [S, C], F32, kind="ExternalOutput")

    with tile.TileContext(nc) as tc:
        with (
            tc.tile_pool(name="consts", bufs=1) as consts,
            tc.tile_pool(name="xpool", bufs=1) as xpool,
            tc.tile_pool(name="wpool", bufs=1) as wpool,
            tc.tile_pool(name="ppool", bufs=1) as ppool,
            tc.tile_pool(name="vtpool", bufs=1) as vtpool,
            tc.tile_pool(name="expool", bufs=18) as expool,
            tc.tile_pool(name="asmpool", bufs=1) as asmpool,
            tc.tile_pool(name="rcppool", bufs=4) as rcppool,
            tc.tile_pool(name="ps", bufs=2, space="PSUM") as ps,
        ):
            # ---- PE warm-up: keep the tensor engine ramping from t~0 so the
            # real projections run at full clock.
            wdum = consts.tile([128, 512], F16, name="wdum")
            nc.vector.memset(wdum, 0.0)
            psdum = ps.tile([128, 512], F32, name="psdum", tag="at")
            for _ in range(N_WARM):
                nc.tensor.matmul(psdum, lhsT=wdum[:, 0:128], rhs=wdum,
                                 start=True, stop=True)
            junk = consts.tile([128, 1], F32, name="junk")
            nc.vector.tensor_copy(out=junk, in_=psdum[:, 0:1])

            # ---- DMAs.  SP and ACT both issue HWDGE DMAs (ACT is idle
            # until the first exp); transfers serialize at HBM bandwidth.
            # SP carries the q-side, ACT the k-side h0 halves.
            bq_t = consts.tile([128, NJ], F32, name="bq_t")
            nc.sync.dma_start(out=bq_t, in_=bqd[:])
            wq_t = wpool.tile([128, NJ, NKC, 128], F16, name="wq_t")
            wk_t = wpool.tile([128, NJ, NKC, 128], F16, name="wk_t")
            nc.sync.dma_start(out=wq_t[:, 0], in_=wq[0])
            nc.scalar.dma_start(out=wk_t[:, 0], in_=wk[0])
            xqk_t = xpool.tile([128, 2 * NKC, S], F16, name="xqk_t")
            for kc in range(NKC):  # q chunks on SP
                nc.sync.dma_start(out=xqk_t[:, kc], in_=xqk[:, kc])
            for kc in range(NKC):  # k chunks s-half 0 on ACT
                nc.scalar.dma_start(out=xqk_t[:, NKC + kc, 0:512],
                                    in_=xqk[:, NKC + kc, 0:512])
            for kc in range(NKC):  # k chunks s-half 1 on SP, after xq
                nc.sync.dma_start(out=xqk_t[:, NKC + kc, 512:1024],
                                  in_=xqk[:, NKC + kc, 512:1024])
            for j in range(1, NJ):
                nc.sync.dma_start(out=wq_t[:, j], in_=wq[j])
                nc.sync.dma_start(out=wk_t[:, j], in_=wk[j])
            wv_t = wpool.tile([128, NKC, C], F16, name="wv_t")
            nc.sync.dma_start(out=wv_t, in_=wv[:])
            xv_t = xpool.tile([128, NKC, S], F16, name="xv_t")
            for kc in range(NKC):
                nc.sync.dma_start(out=xv_t[:, kc], in_=xv[:, kc])
            bvb_t = consts.tile([128, C], F32, name="bvb_t")
            nc.sync.dma_start(out=bvb_t, in_=bvb[:])

            # ---- V' scaffolding: ones columns (cols 64 / 129 per j-pair).
            vt_all = vtpool.tile([128, TCHUNK, NJ, 130], F16, name="vt_all")
            ones32 = consts.tile([128, TCHUNK, NJ], F16, name="ones32")
            nc.gpsimd.memset(ones32, 1.0)
            nc.gpsimd.tensor_copy(out=vt_all[:, :, :, 64], in_=ones32)
            nc.gpsimd.tensor_copy(out=vt_all[:, :, :, 129], in_=ones32)

            pq_ = {}
            pk_ = {}
            for j in range(NJ):
                pq_[j] = ppool.tile([128, S], F16, name=f"pq{j}")
                pk_[j] = ppool.tile([128, S], F16, name=f"pk{j}")

            def proj_half_mms(nm, j, h2):
                """Single-matmul work units for one [128, 512] projection
                accumulator; the last unit also emits the eviction."""
                acc = {}
                w_t = wq_t if nm == "q" else wk_t

                def unit(kc):
                    def go():
                        if kc == 0:
                            acc["t"] = ps.tile([128, 512], F32,
                                               name=f"p{nm}{j}_{h2}", tag="at")
                        nc.tensor.matmul(
                            acc["t"],
                            lhsT=w_t[:, j, kc, :],
                            rhs=xqk_t[:, (0 if nm == "q" else NKC) + kc,
                                      h2 * 512:(h2 + 1) * 512],
                            start=(kc == 0), stop=(kc == NKC - 1),
                        )
                        if kc == NKC - 1:
                            dst = (pq_ if nm == "q" else pk_)[j][
                                :, h2 * 512:(h2 + 1) * 512]
                            if nm == "q":
                                nc.vector.tensor_scalar(
                                    out=dst, in0=acc["t"],
                                    scalar1=bq_t[:, j:j + 1],
                                    scalar2=None, op0=mybir.AluOpType.add)
                            else:
                                nc.vector.tensor_copy(out=dst, in_=acc["t"])
                    return go
                return [unit(kc) for kc in range(NKC)]

            def vacc_mms(tcn):
                """Single-matmul units for the V'^T [t-chunk, c] projection."""
                acc = {}

                def unit(kc):
                    def go():
                        if kc == 0:
                            acc["t"] = ps.tile([128, C], F32,
                                               name=f"vacc{tcn}", tag="at")
                        nc.tensor.matmul(
                            acc["t"],
                            lhsT=xv_t[:, kc, tcn * 128:(tcn + 1) * 128],
                            rhs=wv_t[:, kc, :],
                            start=(kc == 0), stop=(kc == NKC - 1),
                        )
                        if kc == NKC - 1:
                            # V'' = V + bv: the bias rides the projection --
                            # sum_t a (v+bv) / rowsum == out + bv exactly.
                            dst = vt_all[:, tcn, :, :].rearrange(
                                "p j (g d) -> p j g d", g=2)[:, :, :, 0:64]
                            nc.vector.tensor_tensor(
                                out=dst,
                                in0=acc["t"].rearrange(
                                    "p (j g d) -> p j g d", j=NJ, g=2),
                                in1=bvb_t.rearrange(
                                    "p (j g d) -> p j g d", j=NJ, g=2),
                                op=mybir.AluOpType.add)
                    return go
                return [unit(kc) for kc in range(NKC)]

            def proj_half(nm, j, h2):
                for u in proj_half_mms(nm, j, h2):
                    u()

            # ---- pre-attention: j0 projections (q first: its DMA lands
            # first; k's h0 half unblocks QK(0, c<4); k's h1 half is
            # sprinkled at the front of the queue (QK(0, c>=4) needs it).
            proj_half("q", 0, 0)
            proj_half("q", 0, 1)
            proj_half("k", 0, 0)

            # Sprinkled work queue at single-matmul granularity, ordered by
            # DMA readiness and consumer deadlines: k-j0-h1, j1 (used by
            # head 2), then the v projections (xv/wv land ~15us; vacc(tc)
            # must be emitted before AV(0, tc) fires in head-1 block tc),
            # then j2/j3.
            work_q = []
            work_q.extend(proj_half_mms("k", 0, 1))
            work_q.extend(proj_half_mms("q", 1, 0))
            work_q.extend(proj_half_mms("q", 1, 1))
            work_q.extend(proj_half_mms("k", 1, 0))
            for tcn in range(TCHUNK):
                work_q.extend(vacc_mms(tcn))
            # k-j1-h1 is only needed by QK(2, c>=4); it can trail the vaccs
            work_q.extend(proj_half_mms("k", 1, 1))
            for j in (2, 3):
                for nm in ("q", "k"):
                    for h2 in range(2):
                        work_q.extend(proj_half_mms(nm, j, h2))
            # max sprinkled matmuls per (head, c) block (~213ns each against
            # the 1038ns/tile ACT pace)
            SPR_CAP = [3, 3, 2, 2, 2, 2, 1, 0]

            # ---- attention ----
            asm = asmpool.tile([128, TCHUNK, C], F32, name="asm")
            out_r = out.rearrange("(t p) c -> p t c", p=128)
            expt = {}     # (head % 3, c) -> tile
            av_q = {}     # head -> [quad0, quad1]
            sc_tiles = {}

            def emit_qk(i):
                head, c = divmod(i, TCHUNK)
                j, half = head // 2, head % 2
                rows = slice(half * 64, half * 64 + 64)
                sc_t = ps.tile([128, S], F32, name=f"sc{head}_{c}", tag="a")
                for h2 in range(2):
                    hs = slice(h2 * 512, (h2 + 1) * 512)
                    nc.tensor.matmul(
                        sc_t[:, hs],
                        lhsT=pk_[j][rows, c * 128:(c + 1) * 128],
                        rhs=pq_[j][rows, hs],
                        start=True, stop=True,
                    )
                sc_tiles[i] = sc_t

            def emit_exp(i):
                head, c = divmod(i, TCHUNK)
                e = expool.tile([128, S], F16, name=f"e{head}_{c}", tag="pt")
                sc_t = sc_tiles.pop(i)
                if i == NH * TCHUNK - 1:
                    # split the last exp tile so head-7's quad0 AV/finalize/
                    # DMA overlaps the second half (shorter serial tail)
                    for h2 in range(2):
                        hs = slice(h2 * 512, (h2 + 1) * 512)
                        nc.scalar.activation(
                            e[:, hs], sc_t[:, hs],
                            mybir.ActivationFunctionType.Exp, scale=0.125)
                else:
                    nc.scalar.activation(e, sc_t,
                                         mybir.ActivationFunctionType.Exp,
                                         scale=0.125)
                expt[head % 3, c] = e

            def emit_av(head, tcn):
                """8 matmuls: accumulate t-chunk tcn of head's AV."""
                j, half = head // 2, head % 2
                if head not in av_q:
                    # head 7 lag-rides its own head; its quads go on the
                    # (long-free) "at" tag so the av-tag slot FIFO (still
                    # held by head 5/6 until their finalize) can't stall it.
                    avtag = "at" if head == NH - 1 else "av"
                    av_q[head] = [
                        ps.tile([128, 512], F32, name=f"av{head}_{q}",
                                tag=avtag)
                        for q in range(2)]
                e = expt[head % 3, tcn]
                vcols = slice(half * 65, half * 65 + 65)
                for sc_i in range(TCHUNK):
                    quad = av_q[head][sc_i // 4]
                    off = (sc_i % 4) * 128
                    # start=True clears has_written for the WHOLE bank, so
                    # only the first slice of each quad may use it; later
                    # slices' first writes overwrite-and-set per element.
                    nc.tensor.matmul(
                        quad[:, off:off + 65],
                        lhsT=e[:, sc_i * 128:(sc_i + 1) * 128],
                        rhs=vt_all[:, tcn, j, vcols],
                        start=(tcn == 0 and sc_i % 4 == 0),
                        stop=(tcn == TCHUNK - 1),
                        skip_group_check=True,
                    )

            def emit_fin(head, fast_tail=False):
                hs = slice(head * D, (head + 1) * D)
                j = head // 2
                cs = slice(j * 128, (j + 1) * 128)
                for q in range(2):
                    quad = av_q[head][q].rearrange("p (s x) -> p s x", s=4)
                    rcp = rcppool.tile([128, 4], F32, name=f"rcp{head}_{q}")
                    nc.vector.reciprocal(rcp, quad[:, :, 64])
                    qs = slice(q * 4, (q + 1) * 4)
                    nc.vector.tensor_tensor(
                        out=asm[:, qs, hs],
                        in0=quad[:, :, 0:64],
                        in1=rcp.unsqueeze(2).broadcast_to((128, 4, 64)),
                        op=mybir.AluOpType.mult,
                    )
                    if fast_tail:
                        # quarter-DMA right after each quad finalizes (head
                        # 6's cols were finalized just before).
                        nc.sync.dma_start(out=out_r[:, qs, cs],
                                          in_=asm[:, qs, cs])
                if not fast_tail and head % 2 == 1:
                    nc.sync.dma_start(out=out_r[:, :, cs], in_=asm[:, :, cs])

            # Eager-QK pipeline: QK(i+1) is emitted in block i so the exp
            # stream is insulated from AV/sprinkle work by a full tile.
            emit_qk(0)
            for i in range(NH * TCHUNK):
                head, c = divmod(i, TCHUNK)
                if i + 1 < NH * TCHUNK:
                    emit_qk(i + 1)
                emit_exp(i)
                # sprinkles before AV: keeps vacc(tc) ahead of AV(0, tc)
                for _ in range(min(SPR_CAP[head], len(work_q))):
                    work_q.pop(0)()
                if head > 0:
                    emit_av(head - 1, c)          # AV(h-1) rides head h
                if head == NH - 1 and c > 0:
                    emit_av(NH - 1, c - 1)        # AV(7), lag 1
                if head > 0 and c == TCHUNK - 1:
                    emit_fin(head - 1)            # fin 0..6
            # tail
            emit_av(NH - 1, TCHUNK - 1)
            emit_fin(NH - 1, fast_tail=True)

    nc.compile()
    return nc


def _get_nc():
    if "nc" not in _CACHE:
        _CACHE["nc"] = _build()
    return _CACHE["nc"]


def build_in_maps(inputs):
    query, key, value = inputs["query"], inputs["key"], inputs["value"]
    f = np.float32
    B = query.shape[0]

    def pack_w(w):
        # [NJ, 128(p=c_in%128), NKC, 128(m)]: w[j*128+m, kc*128+p]
        wa = np.asarray(w, dtype=f).astype(np.float16)
        wa = wa.reshape(NJ, 128, NKC, 128)        # [j, m, kc, p]
        return np.ascontiguousarray(wa.transpose(0, 3, 2, 1))

    def pack_wv(w):
        wa = np.asarray(w, dtype=f).astype(np.float16)  # [c_out, c_in]
        wa = wa.T.reshape(NKC, 128, C)            # [kc, p, c_out]
        return np.ascontiguousarray(wa.transpose(1, 0, 2))

    def pack_x(x):
        # [C, S] -> [128, NKC, S]
        xa = np.asarray(x, dtype=f).reshape(NKC, 128, S).astype(np.float16)
        return np.ascontiguousarray(xa.transpose(1, 0, 2))

    wq_p = pack_w(inputs["wq"])
    wk_p = pack_w(inputs["wk"])
    wv_p = pack_wv(inputs["wv"])
    bq_p = np.ascontiguousarray(
        np.asarray(inputs["bq"], dtype=f).reshape(NJ, 128).T)
    bvb_p = np.ascontiguousarray(
        np.broadcast_to(np.asarray(inputs["bv"], dtype=f)[None, :], (128, C)))

    in_maps = []
    for b in range(B):
        xq_p = pack_x(np.asarray(query[b], dtype=f).reshape(C, S))
        xk_p = pack_x(np.asarray(key[b], dtype=f).reshape(C, S))
        xv_p = pack_x(np.asarray(value[b], dtype=f).reshape(C, S))
        in_maps.append({
            "xqk": np.ascontiguousarray(
                np.concatenate([xq_p, xk_p], axis=1)),
            "xv": xv_p,
            "wq": wq_p, "wk": wk_p, "wv": wv_p,
            "bqd": bq_p, "bvb": bvb_p,
        })
    return in_maps


def kernel(query, key, value, wq, bq, wk, bk, wv, bv):
    nc = _get_nc()
    B = query.shape[0]
    assert B == N_CORES

    in_maps = build_in_maps({
        "query": query, "key": key, "value": value,
        "wq": wq, "bq": bq, "wk": wk, "bk": bk, "wv": wv, "bv": bv,
    })

    res = bass_utils.run_bass_kernel_spmd(nc, in_maps, core_ids=list(range(B)))
    _CACHE["last_result"] = res
    outs = [res.results[b]["out"].reshape(C, 32, 32) for b in range(B)]
    return np.stack(outs).astype(np.float32)


# revision 24
# speedup vs baseline: 1.0333x; 1.0333x over previous
"""Trainium2 Bass kernel for nn_MultiHeadCrossAttention (v2).

Problem: B=8, C=512, H=W=32 (S=1024), 8 heads x d=64.
Sharding: data-parallel, one batch element per NeuronCore, no collectives.

v2 design (all-fp16 operands, fp32 PSUM accumulation):
  - Bias algebra: softmax(  (q+bq).(k+bk) ) == softmax( (q+bq).k ) since the
    bk cross-term depends only on the query pixel (cancels in the softmax
    normalization) -- bk is dropped entirely; bq is folded into the Q
    projection eviction; bv is re-added at finalize from a host-replicated
    [128, 512] table.
  - Projections q,k in [c, s] layout (P tiles, fp16, k without bias); v is
    projected directly in transposed [t, c] layout (V' tiles per head carry
    a ones column so the AV matmul emits softmax row sums for free).
  - QK computed transposed (scoresT[t, s]) into [128, 1024] PSUM; exp on ACT
    (scale=0.125, no bias) -> fp16 expt tiles.
  - AV *reoriented*: out[s-chunk, 65] accumulates over t-chunks with
    lhsT = expt[:, sc-cols] (stationary, free ldweights) and rhs = V'_head
    [128, 65] (moving).  Output lands directly in [s, head-col] layout:
    no PE transposes, no O evictions.  Two [128, 512] PSUM quads per head
    hold the 8 s-chunks at 128-col offsets.
  - Finalize per head: DVE reciprocal of the rowsum columns + per-quad
    broadcast multiply into the fp32 assembly tile; GPSIMD adds bv in place
    (SBUF-only, keeps DVE/ACT free).  Output DMA per head pair.
  - Schedule: PE warm-up matmuls at t~0; DMA ordered q-side -> k-side with
    chunked transfers so the first exp fires as early as possible; the
    j1..j3 / v projections are sprinkled into the head 0-1 attention stream;
    AV(h) rides head h+1 (AV(7) rides head 7 with lag 1 + a small tail).
  - ACT (exp: 64 x [128,1024] tiles) is the modeled bottleneck engine.
"""

import numpy as np

import concourse.bass as bass  # noqa: F401
import concourse.mybir as mybir
import concourse.tile as tile
from concourse import bacc, bass_utils

F32 = mybir.dt.float32
F16 = mybir.dt.float16

C = 512
S = 1024
NH = 8
D = 64
NKC = C // 128      # 4 contraction chunks
NJ = C // 128       # 4 output chunks (head pairs)
TCHUNK = S // 128   # 8 t-chunks / s-chunks
N_CORES = 8
N_WARM = 6          # PE warm-up matmuls

_CACHE = {}


def _build():
    nc = bacc.Bacc()

    # chunks 0..3: xq kc-chunks; 4..7: xk kc-chunks
    xqk = nc.dram_tensor("xqk", [128, 2 * NKC, S], F16, kind="ExternalInput")
    xv = nc.dram_tensor("xv", [128, NKC, S], F16, kind="ExternalInput")
    # [j][p = c_in % 128][kc][m = c_out - j*128]
    wq = nc.dram_tensor("wq", [NJ, 128, NKC, 128], F16, kind="ExternalInput")
    wk = nc.dram_tensor("wk", [NJ, 128, NKC, 128], F16, kind="ExternalInput")
    # [p = c_in % 128][kc][c_out]
    wv = nc.dram_tensor("wv", [128, NKC, C], F16, kind="ExternalInput")
    bqd = nc.dram_tensor("bqd", [128, NJ], F32, kind="ExternalInput")
    bvb = nc.dram_tensor("bvb", [128, C], F32, kind="ExternalInput")
    out = nc.dram_tensor("out", [S, C], F32, kind="ExternalOutput")

    with tile.TileContext(nc) as tc:
        with (
            tc.tile_pool(name="consts", bufs=1) as consts,
            tc.tile_pool(name="xpool", bufs=1) as xpool,
            tc.tile_pool(name="wpool", bufs=1) as wpool,
            tc.tile_pool(name="ppool", bufs=1) as ppool,
            tc.tile_pool(name="vtpool", bufs=1) as vtpool,
            tc.tile_pool(name="expool", bufs=18) as expool,
            tc.tile_pool(name="asmpool", bufs=1) as asmpool,
            tc.tile_pool(name="rcppool", bufs=4) as rcppool,
            tc.tile_pool(name="ps", bufs=2, space="PSUM") as ps,
        ):
            # ---- PE warm-up: keep the tensor engine ramping from t~0 so the
            # real projections run at full clock.
            wdum = consts.tile([128, 512], F16, name="wdum")
            nc.vector.memset(wdum, 0.0)
            psdum = ps.tile([128, 512], F32, name="psdum", tag="at")
            for _ in range(N_WARM):
                nc.tensor.matmul(psdum, lhsT=wdum[:, 0:128], rhs=wdum,
                                 start=True, stop=True)
            junk = consts.tile([128, 1], F32, name="junk")
            nc.vector.tensor_copy(out=junk, in_=psdum[:, 0:1])

            # ---- DMAs, all on SP in strict priority order (transfers
            # serialize at HBM bandwidth; q-side first so the q projection
            # streams behind its chunks, then k-h0 which gates QK(0,0)).
            bq_t = consts.tile([128, NJ], F32, name="bq_t")
            nc.sync.dma_start(out=bq_t, in_=bqd[:])
            wq_t = wpool.tile([128, NJ, NKC, 128], F16, name="wq_t")
            wk_t = wpool.tile([128, NJ, NKC, 128], F16, name="wk_t")
            nc.sync.dma_start(out=wq_t[:, 0], in_=wq[0])
            xqk_t = xpool.tile([128, 2 * NKC, S], F16, name="xqk_t")
            for kc in range(NKC):  # q chunks
                nc.sync.dma_start(out=xqk_t[:, kc], in_=xqk[:, kc])
            nc.sync.dma_start(out=wk_t[:, 0], in_=wk[0])
            for kc in range(NKC):  # k chunks, s-half 0 (gates QK(0, c<4))
                nc.sync.dma_start(out=xqk_t[:, NKC + kc, 0:512],
                                  in_=xqk[:, NKC + kc, 0:512])
            for kc in range(NKC):
                nc.sync.dma_start(out=xqk_t[:, NKC + kc, 512:1024],
                                  in_=xqk[:, NKC + kc, 512:1024])
            wv_t = wpool.tile([128, NKC, C], F16, name="wv_t")
            nc.sync.dma_start(out=wv_t, in_=wv[:])
            xv_t = xpool.tile([128, NKC, S], F16, name="xv_t")
            for kc in range(NKC):
                nc.sync.dma_start(out=xv_t[:, kc], in_=xv[:, kc])
            bvb_t = consts.tile([128, C], F32, name="bvb_t")
            nc.sync.dma_start(out=bvb_t, in_=bvb[:])
            for j in range(1, NJ):
                nc.sync.dma_start(out=wq_t[:, j], in_=wq[j])
                nc.sync.dma_start(out=wk_t[:, j], in_=wk[j])

            # ---- V' scaffolding: ones columns (cols 64 / 129 per j-pair).
            vt_all = vtpool.tile([128, TCHUNK, NJ, 130], F16, name="vt_all")
            ones32 = consts.tile([128, TCHUNK, NJ], F16, name="ones32")
            nc.gpsimd.memset(ones32, 1.0)
            nc.gpsimd.tensor_copy(out=vt_all[:, :, :, 64], in_=ones32)
            nc.gpsimd.tensor_copy(out=vt_all[:, :, :, 129], in_=ones32)

            pq_ = {}
            pk_ = {}
            for j in range(NJ):
                pq_[j] = ppool.tile([128, S], F16, name=f"pq{j}")
                pk_[j] = ppool.tile([128, S], F16, name=f"pk{j}")

            def proj_half_mms(nm, j, h2, split_evict=False):
                """Single-matmul work units for one [128, 512] projection
                accumulator; the last unit also emits the eviction (for the
                QK(0,0)-critical k-j0-h0 half, the c0 chunk is evicted
                separately so the first QK unblocks early)."""
                acc = {}
                w_t = wq_t if nm == "q" else wk_t

                def unit(kc):
                    def go():
                        if kc == 0:
                            acc["t"] = ps.tile([128, 512], F32,
                                               name=f"p{nm}{j}_{h2}", tag="at")
                        nc.tensor.matmul(
                            acc["t"],
                            lhsT=w_t[:, j, kc, :],
                            rhs=xqk_t[:, (0 if nm == "q" else NKC) + kc,
                                      h2 * 512:(h2 + 1) * 512],
                            start=(kc == 0), stop=(kc == NKC - 1),
                        )
                        if kc == NKC - 1:
                            dst = (pq_ if nm == "q" else pk_)[j][
                                :, h2 * 512:(h2 + 1) * 512]
                            if nm == "q":
                                nc.vector.tensor_scalar(
                                    out=dst, in0=acc["t"],
                                    scalar1=bq_t[:, j:j + 1],
                                    scalar2=None, op0=mybir.AluOpType.add)
                            elif split_evict:
                                nc.vector.tensor_copy(
                                    out=dst[:, 0:128], in_=acc["t"][:, 0:128])
                                nc.vector.tensor_copy(
                                    out=dst[:, 128:512],
                                    in_=acc["t"][:, 128:512])
                            else:
                                nc.vector.tensor_copy(out=dst, in_=acc["t"])
                    return go
                return [unit(kc) for kc in range(NKC)]

            def vacc_mms(tcn, j):
                """Single-matmul units for head-pair j's slice of the V'^T
                [t-chunk, c] projection (out [128, 128]; 53ns each)."""
                acc = {}
                js = slice(j * 128, (j + 1) * 128)

                def unit(kc):
                    def go():
                        if kc == 0:
                            acc["t"] = ps.tile([128, 128], F32,
                                               name=f"vacc{tcn}_{j}",
                                               tag="at")
                        nc.tensor.matmul(
                            acc["t"],
                            lhsT=xv_t[:, kc, tcn * 128:(tcn + 1) * 128],
                            rhs=wv_t[:, kc, js],
                            start=(kc == 0), stop=(kc == NKC - 1),
                        )
                        if kc == NKC - 1:
                            # V'' = V + bv: the bias rides the projection --
                            # sum_t a (v+bv) / rowsum == out + bv exactly.
                            dst = vt_all[:, tcn, j, :].rearrange(
                                "p (g d) -> p g d", g=2)[:, :, 0:64]
                            nc.vector.tensor_tensor(
                                out=dst,
                                in0=acc["t"].rearrange(
                                    "p (g d) -> p g d", g=2),
                                in1=bvb_t[:, js].rearrange(
                                    "p (g d) -> p g d", g=2),
                                op=mybir.AluOpType.add)
                    return go
                return [unit(kc) for kc in range(NKC)]

            def proj_half(nm, j, h2, split_evict=False):
                for u in proj_half_mms(nm, j, h2, split_evict):
                    u()

            # ---- pre-attention: j0 projections (q first: its DMA lands
            # first; k's h0 half unblocks QK(0, c<4); k's h1 half is
            # sprinkled with an early deadline).
            proj_half("q", 0, 0)
            proj_half("q", 0, 1)
            proj_half("k", 0, 0, split_evict=True)

            # Sprinkled work queue at single-matmul granularity.  Each unit
            # carries a deadline (global block index by which it must be
            # EMITTED: its consumer matmul follows it in PE program order).
            # Units are kept deadline-sorted; each block force-pops due
            # units and greedily tops up to a small cap.
            work_q = []   # (deadline, ready, closure)

            def add_units(units, deadline, ready=0):
                for u in units:
                    work_q.append((deadline, ready, u))

            add_units(proj_half_mms("k", 0, 1), 2)        # QK(0,4) @ blk 3
            add_units(proj_half_mms("q", 1, 0), 13, 5)    # QK(2,0) @ blk 15
            add_units(proj_half_mms("q", 1, 1), 13, 5)
            add_units(proj_half_mms("k", 1, 0), 14, 5)
            add_units(proj_half_mms("k", 1, 1), 18, 5)    # QK(2,4) @ blk 19
            for j in range(NJ):                           # v: AV(2j, tc)
                for tcn in range(TCHUNK):                 # @ blk (2j+1)*8+tc
                    # cap at 54: all v units must precede the av7 quads in
                    # the at-tag slot FIFO (allocated at head-7 block 57)
                    add_units(vacc_mms(tcn, j),
                              min((2 * j + 1) * 8 + tcn - 1, 54), 5)
            add_units(proj_half_mms("q", 2, 0), 29, 5)    # QK(4,0) @ blk 31
            add_units(proj_half_mms("q", 2, 1), 29, 5)
            add_units(proj_half_mms("k", 2, 0), 30, 5)
            add_units(proj_half_mms("k", 2, 1), 34, 5)    # QK(4,4) @ blk 35
            add_units(proj_half_mms("q", 3, 0), 45, 5)    # QK(6,0) @ blk 47
            add_units(proj_half_mms("q", 3, 1), 45, 5)
            add_units(proj_half_mms("k", 3, 0), 46, 5)
            add_units(proj_half_mms("k", 3, 1), 50, 5)    # QK(6,4) @ blk 51
            work_q.sort(key=lambda t: t[0])
            SPR_CAP = 2

            # ---- attention ----
            asm = asmpool.tile([128, TCHUNK, C], F32, name="asm")
            out_r = out.rearrange("(t p) c -> p t c", p=128)
            expt = {}     # (head % 3, c) -> tile
            av_q = {}     # head -> [quad0, quad1]
            sc_tiles = {}

            def emit_qk(i):
                head, c = divmod(i, TCHUNK)
                j, half = head // 2, head % 2
                rows = slice(half * 64, half * 64 + 64)
                sc_t = ps.tile([128, S], F32, name=f"sc{head}_{c}", tag="a")
                for h2 in range(2):
                    hs = slice(h2 * 512, (h2 + 1) * 512)
                    nc.tensor.matmul(
                        sc_t[:, hs],
                        lhsT=pk_[j][rows, c * 128:(c + 1) * 128],
                        rhs=pq_[j][rows, hs],
                        start=True, stop=True,
                    )
                sc_tiles[i] = sc_t

            def emit_exp(i):
                head, c = divmod(i, TCHUNK)
                e = expool.tile([128, S], F16, name=f"e{head}_{c}", tag="pt")
                sc_t = sc_tiles.pop(i)
                if i == NH * TCHUNK - 1:
                    # split the last exp tile so head-7's quad0 AV/finalize/
                    # DMA overlaps the second half (shorter serial tail)
                    for h2 in range(2):
                        hs = slice(h2 * 512, (h2 + 1) * 512)
                        nc.scalar.activation(
                            e[:, hs], sc_t[:, hs],
                            mybir.ActivationFunctionType.Exp, scale=0.125)
                else:
                    nc.scalar.activation(e, sc_t,
                                         mybir.ActivationFunctionType.Exp,
                                         scale=0.125)
                expt[head % 3, c] = e

            def emit_av(head, tcn):
                """8 matmuls: accumulate t-chunk tcn of head's AV."""
                j, half = head // 2, head % 2
                if head not in av_q:
                    # head 7 lag-rides its own head; its quads go on the
                    # (long-free) "at" tag so the av-tag slot FIFO (still
                    # held by head 5/6 until their finalize) can't stall it.
                    avtag = "at" if head == NH - 1 else "av"
                    av_q[head] = [
                        ps.tile([128, 512], F32, name=f"av{head}_{q}",
                                tag=avtag)
                        for q in range(2)]
                e = expt[head % 3, tcn]
                vcols = slice(half * 65, half * 65 + 65)
                for sc_i in range(TCHUNK):
                    quad = av_q[head][sc_i // 4]
                    off = (sc_i % 4) * 128
                    # start=True clears has_written for the WHOLE bank, so
                    # only the first slice of each quad may use it; later
                    # slices' first writes overwrite-and-set per element.
                    nc.tensor.matmul(
                        quad[:, off:off + 65],
                        lhsT=e[:, sc_i * 128:(sc_i + 1) * 128],
                        rhs=vt_all[:, tcn, j, vcols],
                        start=(tcn == 0 and sc_i % 4 == 0),
                        stop=(tcn == TCHUNK - 1),
                        skip_group_check=True,
                    )

            def emit_fin(head, fast_tail=False):
                hs = slice(head * D, (head + 1) * D)
                j = head // 2
                cs = slice(j * 128, (j + 1) * 128)
                for q in range(2):
                    quad = av_q[head][q].rearrange("p (s x) -> p s x", s=4)
                    rcp = rcppool.tile([128, 4], F32, name=f"rcp{head}_{q}")
                    nc.vector.reciprocal(rcp, quad[:, :, 64])
                    qs = slice(q * 4, (q + 1) * 4)
                    nc.vector.tensor_tensor(
                        out=asm[:, qs, hs],
                        in0=quad[:, :, 0:64],
                        in1=rcp.unsqueeze(2).broadcast_to((128, 4, 64)),
                        op=mybir.AluOpType.mult,
                    )
                    if fast_tail:
                        # quarter-DMA right after each quad finalizes (head
                        # 6's cols were finalized just before).
                        nc.sync.dma_start(out=out_r[:, qs, cs],
                                          in_=asm[:, qs, cs])
                if not fast_tail and head % 2 == 1:
                    nc.sync.dma_start(out=out_r[:, :, cs], in_=asm[:, :, cs])

            # Eager-QK pipeline: QK(i+1) is emitted in block i so the exp
            # stream is insulated from AV/sprinkle work by a full tile.
            emit_qk(0)
            for i in range(NH * TCHUNK):
                head, c = divmod(i, TCHUNK)
                if i + 1 < NH * TCHUNK:
                    emit_qk(i + 1)
                emit_exp(i)
                # sprinkles before AV: force-pop due units; top up with
                # DMA-ready ones
                n_pop = 0
                while work_q and (work_q[0][0] <= i or
                                  (work_q[0][1] <= i and n_pop < SPR_CAP)):
                    work_q.pop(0)[2]()
                    n_pop += 1
                if head > 0:
                    emit_av(head - 1, c)          # AV(h-1) rides head h
                if head == NH - 1 and c > 0:
                    emit_av(NH - 1, c - 1)        # AV(7), lag 1
                if head > 0 and c == TCHUNK - 1:
                    emit_fin(head - 1)            # fin 0..6
            # tail
            emit_av(NH - 1, TCHUNK - 1)
            emit_fin(NH - 1, fast_tail=True)

    nc.compile()
    return nc


def _get_nc():
    if "nc" not in _CACHE:
        _CACHE["nc"] = _build()
    return _CACHE["nc"]


def build_in_maps(inputs):
    query, key, value = inputs["query"], inputs["key"], inputs["value"]
    f = np.float32
    B = query.shape[0]

    def pack_w(w):
        # [NJ, 128(p=c_in%128), NKC, 128(m)]: w[j*128+m, kc*128+p]
        wa = np.asarray(w, dtype=f).astype(np.float16)
        wa = wa.reshape(NJ, 128, NKC, 128)        # [j, m, kc, p]
        return np.ascontiguousarray(wa.transpose(0, 3, 2, 1))

    def pack_wv(w):
        wa = np.asarray(w, dtype=f).astype(np.float16)  # [c_out, c_in]
        wa = wa.T.reshape(NKC, 128, C)            # [kc, p, c_out]
        return np.ascontiguousarray(wa.transpose(1, 0, 2))

    def pack_x(x):
        # [C, S] -> [128, NKC, S]
        xa = np.asarray(x, dtype=f).reshape(NKC, 128, S).astype(np.float16)
        return np.ascontiguousarray(xa.transpose(1, 0, 2))

    wq_p = pack_w(inputs["wq"])
    wk_p = pack_w(inputs["wk"])
    wv_p = pack_wv(inputs["wv"])
    bq_p = np.ascontiguousarray(
        np.asarray(inputs["bq"], dtype=f).reshape(NJ, 128).T)
    bvb_p = np.ascontiguousarray(
        np.broadcast_to(np.asarray(inputs["bv"], dtype=f)[None, :], (128, C)))

    in_maps = []
    for b in range(B):
        xq_p = pack_x(np.asarray(query[b], dtype=f).reshape(C, S))
        xk_p = pack_x(np.asarray(key[b], dtype=f).reshape(C, S))
        xv_p = pack_x(np.asarray(value[b], dtype=f).reshape(C, S))
        in_maps.append({
            "xqk": np.ascontiguousarray(
                np.concatenate([xq_p, xk_p], axis=1)),
            "xv": xv_p,
            "wq": wq_p, "wk": wk_p, "wv": wv_p,
            "bqd": bq_p, "bvb": bvb_p,
        })
    return in_maps


def kernel(query, key, value, wq, bq, wk, bk, wv, bv):
    nc = _get_nc()
    B = query.shape[0]
    assert B == N_CORES

    in_maps = build_in_maps({
        "query": query, "key": key, "value": value,
        "wq": wq, "bq": bq, "wk": wk, "bk": bk, "wv": wv, "bv": bv,
    })

    res = bass_utils.run_bass_kernel_spmd(nc, in_maps, core_ids=list(range(B)))
    _CACHE["last_result"] = res
    outs = [res.results[b]["out"].reshape(C, 32, 32) for b in range(B)]
    return np.stack(outs).astype(np.float32)


# revision 26
# speedup vs baseline: 1.0517x; 1.0178x over previous
"""Trainium2 Bass kernel for nn_MultiHeadCrossAttention (v2).

Problem: B=8, C=512, H=W=32 (S=1024), 8 heads x d=64.
Sharding: data-parallel, one batch element per NeuronCore, no collectives.

v2 design (all-fp16 operands, fp32 PSUM accumulation):
  - Bias algebra: softmax(  (q+bq).(k+bk) ) == softmax( (q+bq).k ) since the
    bk cross-term depends only on the query pixel (cancels in the softmax
    normalization) -- bk is dropped entirely; bq is folded into the Q
    projection eviction; bv is re-added at finalize from a host-replicated
    [128, 512] table.
  - Projections q,k in [c, s] layout (P tiles, fp16, k without bias); v is
    projected directly in transposed [t, c] layout (V' tiles per head carry
    a ones column so the AV matmul emits softmax row sums for free).
  - QK computed transposed (scoresT[t, s]) into [128, 1024] PSUM; exp on ACT
    (scale=0.125, no bias) -> fp16 expt tiles.
  - AV *reoriented*: out[s-chunk, 65] accumulates over t-chunks with
    lhsT = expt[:, sc-cols] (stationary, free ldweights) and rhs = V'_head
    [128, 65] (moving).  Output lands directly in [s, head-col] layout:
    no PE transposes, no O evictions.  Two [128, 512] PSUM quads per head
    hold the 8 s-chunks at 128-col offsets.
  - Finalize per head: DVE reciprocal of the rowsum columns + per-quad
    broadcast multiply into the fp32 assembly tile; GPSIMD adds bv in place
    (SBUF-only, keeps DVE/ACT free).  Output DMA per head pair.
  - Schedule: PE warm-up matmuls at t~0; DMA ordered q-side -> k-side with
    chunked transfers so the first exp fires as early as possible; the
    j1..j3 / v projections are sprinkled into the head 0-1 attention stream;
    AV(h) rides head h+1 (AV(7) rides head 7 with lag 1 + a small tail).
  - ACT (exp: 64 x [128,1024] tiles) is the modeled bottleneck engine.
"""

import numpy as np

import concourse.bass as bass  # noqa: F401
import concourse.mybir as mybir
import concourse.tile as tile
from concourse import bacc, bass_utils

F32 = mybir.dt.float32
F16 = mybir.dt.float16

C = 512
S = 1024
NH = 8
D = 64
NKC = C // 128      # 4 contraction chunks
NJ = C // 128       # 4 output chunks (head pairs)
TCHUNK = S // 128   # 8 t-chunks / s-chunks
N_CORES = 8
N_WARM = 8          # PE warm-up matmuls

_CACHE = {}


def _build():
    nc = bacc.Bacc()

    # chunks 0..3: xq kc-chunks; 4..7: xk kc-chunks
    xqk = nc.dram_tensor("xqk", [128, 2 * NKC, S], F16, kind="ExternalInput")
    xv = nc.dram_tensor("xv", [128, NKC, S], F16, kind="ExternalInput")
    # [j][p = c_in % 128][kc][m = c_out - j*128]
    wq = nc.dram_tensor("wq", [NJ, 128, NKC, 128], F16, kind="ExternalInput")
    wk = nc.dram_tensor("wk", [NJ, 128, NKC, 128], F16, kind="ExternalInput")
    # [p = c_in % 128][kc][c_out]
    wv = nc.dram_tensor("wv", [128, NKC, C], F16, kind="ExternalInput")
    bqd = nc.dram_tensor("bqd", [128, NJ], F32, kind="ExternalInput")
    bvb = nc.dram_tensor("bvb", [128, C], F32, kind="ExternalInput")
    out = nc.dram_tensor("out", [S, C], F32, kind="ExternalOutput")

    with tile.TileContext(nc) as tc:
        with (
            tc.tile_pool(name="consts", bufs=1) as consts,
            tc.tile_pool(name="xpool", bufs=1) as xpool,
            tc.tile_pool(name="wpool", bufs=1) as wpool,
            tc.tile_pool(name="ppool", bufs=1) as ppool,
            tc.tile_pool(name="vtpool", bufs=1) as vtpool,
            tc.tile_pool(name="expool", bufs=18) as expool,
            tc.tile_pool(name="asmpool", bufs=1) as asmpool,
            tc.tile_pool(name="rcppool", bufs=4) as rcppool,
            tc.tile_pool(name="ps", bufs=2, space="PSUM") as ps,
        ):
            # ---- PE warm-up: keep the tensor engine ramping from t~0 so the
            # real projections run at full clock.
            wdum = consts.tile([128, 512], F16, name="wdum")
            nc.vector.memset(wdum, 0.0)
            psdum = ps.tile([128, 512], F32, name="psdum", tag="at")
            for _ in range(N_WARM):
                nc.tensor.matmul(psdum, lhsT=wdum[:, 0:128], rhs=wdum,
                                 start=True, stop=True)
            junk = consts.tile([128, 1], F32, name="junk")
            nc.vector.tensor_copy(out=junk, in_=psdum[:, 0:1])

            # ---- DMAs, all on SP in strict priority order (transfers
            # serialize at HBM bandwidth; q-side first so the q projection
            # streams behind its chunks, then k-h0 which gates QK(0,0)).
            bq_t = consts.tile([128, NJ], F32, name="bq_t")
            nc.sync.dma_start(out=bq_t, in_=bqd[:])
            wq_t = wpool.tile([128, NJ, NKC, 128], F16, name="wq_t")
            wk_t = wpool.tile([128, NJ, NKC, 128], F16, name="wk_t")
            nc.sync.dma_start(out=wq_t[:, 0], in_=wq[0])
            xqk_t = xpool.tile([128, 2 * NKC, S], F16, name="xqk_t")
            for h in range(2):  # q chunk pairs
                nc.sync.dma_start(out=xqk_t[:, 2 * h:2 * h + 2],
                                  in_=xqk[:, 2 * h:2 * h + 2])
            nc.sync.dma_start(out=wk_t[:, 0], in_=wk[0])
            for h in range(2):  # k chunk pairs, s-half 0 (gates QK(0, c<4))
                ks = slice(NKC + 2 * h, NKC + 2 * h + 2)
                nc.sync.dma_start(out=xqk_t[:, ks, 0:512],
                                  in_=xqk[:, ks, 0:512])
            nc.sync.dma_start(out=xqk_t[:, NKC:, 512:1024],
                              in_=xqk[:, NKC:, 512:1024])
            wv_t = wpool.tile([128, NKC, C], F16, name="wv_t")
            nc.sync.dma_start(out=wv_t, in_=wv[:])
            xv_t = xpool.tile([128, NKC, S], F16, name="xv_t")
            nc.sync.dma_start(out=xv_t, in_=xv[:])
            bvb_t = consts.tile([128, C], F32, name="bvb_t")
            nc.sync.dma_start(out=bvb_t, in_=bvb[:])
            nc.sync.dma_start(out=wq_t[:, 1:], in_=wq[1:].rearrange(
                "j p kc m -> p j kc m"))
            nc.sync.dma_start(out=wk_t[:, 1:], in_=wk[1:].rearrange(
                "j p kc m -> p j kc m"))

            # ---- V' scaffolding: ones columns (cols 64 / 129 per j-pair).
            vt_all = vtpool.tile([128, TCHUNK, NJ, 130], F16, name="vt_all")
            ones32 = consts.tile([128, TCHUNK, NJ], F16, name="ones32")
            nc.gpsimd.memset(ones32, 1.0)
            nc.gpsimd.tensor_copy(out=vt_all[:, :, :, 64], in_=ones32)
            nc.gpsimd.tensor_copy(out=vt_all[:, :, :, 129], in_=ones32)

            pq_ = {}
            pk_ = {}
            for j in range(NJ):
                pq_[j] = ppool.tile([128, S], F16, name=f"pq{j}")
                pk_[j] = ppool.tile([128, S], F16, name=f"pk{j}")

            def proj_half_mms(nm, j, h2, split_evict=False):
                """Single-matmul work units for one [128, 512] projection
                accumulator; the last unit also emits the eviction (for the
                QK(0,0)-critical k-j0-h0 half, the c0 chunk is evicted
                separately so the first QK unblocks early)."""
                acc = {}
                w_t = wq_t if nm == "q" else wk_t

                def unit(kc):
                    def go():
                        if kc == 0:
                            acc["t"] = ps.tile([128, 512], F32,
                                               name=f"p{nm}{j}_{h2}", tag="at")
                        nc.tensor.matmul(
                            acc["t"],
                            lhsT=w_t[:, j, kc, :],
                            rhs=xqk_t[:, (0 if nm == "q" else NKC) + kc,
                                      h2 * 512:(h2 + 1) * 512],
                            start=(kc == 0), stop=(kc == NKC - 1),
                        )
                        if kc == NKC - 1:
                            dst = (pq_ if nm == "q" else pk_)[j][
                                :, h2 * 512:(h2 + 1) * 512]
                            if nm == "q":
                                nc.vector.tensor_scalar(
                                    out=dst, in0=acc["t"],
                                    scalar1=bq_t[:, j:j + 1],
                                    scalar2=None, op0=mybir.AluOpType.add)
                            elif split_evict:
                                nc.vector.tensor_copy(
                                    out=dst[:, 0:128], in_=acc["t"][:, 0:128])
                                nc.vector.tensor_copy(
                                    out=dst[:, 128:512],
                                    in_=acc["t"][:, 128:512])
                            else:
                                nc.vector.tensor_copy(out=dst, in_=acc["t"])
                    return go
                return [unit(kc) for kc in range(NKC)]

            def vacc_mms(tcn, j):
                """Single-matmul units for head-pair j's slice of the V'^T
                [t-chunk, c] projection (out [128, 128]; 53ns each)."""
                acc = {}
                js = slice(j * 128, (j + 1) * 128)

                def unit(kc):
                    def go():
                        if kc == 0:
                            acc["t"] = ps.tile([128, 128], F32,
                                               name=f"vacc{tcn}_{j}",
                                               tag="at")
                        nc.tensor.matmul(
                            acc["t"],
                            lhsT=xv_t[:, kc, tcn * 128:(tcn + 1) * 128],
                            rhs=wv_t[:, kc, js],
                            start=(kc == 0), stop=(kc == NKC - 1),
                        )
                        if kc == NKC - 1:
                            # V'' = V + bv: the bias rides the projection --
                            # sum_t a (v+bv) / rowsum == out + bv exactly.
                            dst = vt_all[:, tcn, j, :].rearrange(
                                "p (g d) -> p g d", g=2)[:, :, 0:64]
                            nc.vector.tensor_tensor(
                                out=dst,
                                in0=acc["t"].rearrange(
                                    "p (g d) -> p g d", g=2),
                                in1=bvb_t[:, js].rearrange(
                                    "p (g d) -> p g d", g=2),
                                op=mybir.AluOpType.add)
                    return go
                return [unit(kc) for kc in range(NKC)]

            def proj_half(nm, j, h2, split_evict=False):
                for u in proj_half_mms(nm, j, h2, split_evict):
                    u()

            # ---- pre-attention: j0 projections (q first: its DMA lands
            # first; k's h0 half unblocks QK(0, c<4); k's h1 half is
            # sprinkled with an early deadline).
            proj_half("q", 0, 0)
            proj_half("q", 0, 1)
            proj_half("k", 0, 0, split_evict=True)

            # Sprinkled work queue at single-matmul granularity.  Each unit
            # carries a deadline (global block index by which it must be
            # EMITTED: its consumer matmul follows it in PE program order).
            # Units are kept deadline-sorted; each block force-pops due
            # units and greedily tops up to a small cap.
            work_q = []   # (deadline, ready, closure)

            def add_units(units, deadline, ready=0):
                for u in units:
                    work_q.append((deadline, ready, u))

            add_units(proj_half_mms("k", 0, 1), 2)        # QK(0,4) @ blk 3
            add_units(proj_half_mms("q", 1, 0), 13, 5)    # QK(2,0) @ blk 15
            add_units(proj_half_mms("q", 1, 1), 13, 5)
            add_units(proj_half_mms("k", 1, 0), 14, 5)
            add_units(proj_half_mms("k", 1, 1), 18, 5)    # QK(2,4) @ blk 19
            for j in range(NJ):                           # v: AV(2j, tc)
                for tcn in range(TCHUNK):                 # @ blk (2j+1)*8+tc
                    # cap at 54: all v units must precede the av7 quads in
                    # the at-tag slot FIFO (allocated at head-7 block 57)
                    add_units(vacc_mms(tcn, j),
                              min((2 * j + 1) * 8 + tcn - 1, 54), 5)
            add_units(proj_half_mms("q", 2, 0), 29, 5)    # QK(4,0) @ blk 31
            add_units(proj_half_mms("q", 2, 1), 29, 5)
            add_units(proj_half_mms("k", 2, 0), 30, 5)
            add_units(proj_half_mms("k", 2, 1), 34, 5)    # QK(4,4) @ blk 35
            add_units(proj_half_mms("q", 3, 0), 45, 5)    # QK(6,0) @ blk 47
            add_units(proj_half_mms("q", 3, 1), 45, 5)
            add_units(proj_half_mms("k", 3, 0), 46, 5)
            add_units(proj_half_mms("k", 3, 1), 50, 5)    # QK(6,4) @ blk 51
            work_q.sort(key=lambda t: t[0])
            SPR_CAP = 2

            # ---- attention ----
            asm = asmpool.tile([128, TCHUNK, C], F32, name="asm")
            out_r = out.rearrange("(t p) c -> p t c", p=128)
            expt = {}     # (head % 3, c) -> tile
            av_q = {}     # head -> [quad0, quad1]
            sc_tiles = {}

            def emit_qk(i):
                head, c = divmod(i, TCHUNK)
                j, half = head // 2, head % 2
                rows = slice(half * 64, half * 64 + 64)
                sc_t = ps.tile([128, S], F32, name=f"sc{head}_{c}", tag="a")
                for h2 in range(2):
                    hs = slice(h2 * 512, (h2 + 1) * 512)
                    nc.tensor.matmul(
                        sc_t[:, hs],
                        lhsT=pk_[j][rows, c * 128:(c + 1) * 128],
                        rhs=pq_[j][rows, hs],
                        start=True, stop=True,
                    )
                sc_tiles[i] = sc_t

            def emit_exp(i):
                head, c = divmod(i, TCHUNK)
                e = expool.tile([128, S], F16, name=f"e{head}_{c}", tag="pt")
                sc_t = sc_tiles.pop(i)
                if i == NH * TCHUNK - 1:
                    # split the last exp tile so head-7's quad0 AV/finalize/
                    # DMA overlaps the second half (shorter serial tail)
                    for h2 in range(2):
                        hs = slice(h2 * 512, (h2 + 1) * 512)
                        nc.scalar.activation(
                            e[:, hs], sc_t[:, hs],
                            mybir.ActivationFunctionType.Exp, scale=0.125)
                else:
                    nc.scalar.activation(e, sc_t,
                                         mybir.ActivationFunctionType.Exp,
                                         scale=0.125)
                expt[head % 3, c] = e

            def emit_av(head, tcn):
                """8 matmuls: accumulate t-chunk tcn of head's AV."""
                j, half = head // 2, head % 2
                if head not in av_q:
                    # head 7 lag-rides its own head; its quads go on the
                    # (long-free) "at" tag so the av-tag slot FIFO (still
                    # held by head 5/6 until their finalize) can't stall it.
                    avtag = "at" if head == NH - 1 else "av"
                    av_q[head] = [
                        ps.tile([128, 512], F32, name=f"av{head}_{q}",
                                tag=avtag)
                        for q in range(2)]
                e = expt[head % 3, tcn]
                vcols = slice(half * 65, half * 65 + 65)
                for sc_i in range(TCHUNK):
                    quad = av_q[head][sc_i // 4]
                    off = (sc_i % 4) * 128
                    # start=True clears has_written for the WHOLE bank, so
                    # only the first slice of each quad may use it; later
                    # slices' first writes overwrite-and-set per element.
                    nc.tensor.matmul(
                        quad[:, off:off + 65],
                        lhsT=e[:, sc_i * 128:(sc_i + 1) * 128],
                        rhs=vt_all[:, tcn, j, vcols],
                        start=(tcn == 0 and sc_i % 4 == 0),
                        stop=(tcn == TCHUNK - 1),
                        skip_group_check=True,
                    )

            def emit_fin(head, fast_tail=False):
                hs = slice(head * D, (head + 1) * D)
                j = head // 2
                cs = slice(j * 128, (j + 1) * 128)
                for q in range(2):
                    quad = av_q[head][q].rearrange("p (s x) -> p s x", s=4)
                    rcp = rcppool.tile([128, 4], F32, name=f"rcp{head}_{q}")
                    nc.vector.reciprocal(rcp, quad[:, :, 64])
                    qs = slice(q * 4, (q + 1) * 4)
                    nc.vector.tensor_tensor(
                        out=asm[:, qs, hs],
                        in0=quad[:, :, 0:64],
                        in1=rcp.unsqueeze(2).broadcast_to((128, 4, 64)),
                        op=mybir.AluOpType.mult,
                    )
                    if fast_tail:
                        # quarter-DMA right after each quad finalizes (head
                        # 6's cols were finalized just before).
                        nc.sync.dma_start(out=out_r[:, qs, cs],
                                          in_=asm[:, qs, cs])
                if not fast_tail and head % 2 == 1:
                    nc.sync.dma_start(out=out_r[:, :, cs], in_=asm[:, :, cs])

            # Eager-QK pipeline: QK(i+1) is emitted in block i so the exp
            # stream is insulated from AV/sprinkle work by a full tile.
            emit_qk(0)
            for i in range(NH * TCHUNK):
                head, c = divmod(i, TCHUNK)
                if i + 1 < NH * TCHUNK:
                    emit_qk(i + 1)
                emit_exp(i)
                # sprinkles before AV: force-pop due units; top up with
                # DMA-ready ones
                n_pop = 0
                while work_q and (work_q[0][0] <= i or
                                  (work_q[0][1] <= i and n_pop < SPR_CAP)):
                    work_q.pop(0)[2]()
                    n_pop += 1
                if head > 0:
                    emit_av(head - 1, c)          # AV(h-1) rides head h
                if head == NH - 1 and c > 0:
                    emit_av(NH - 1, c - 1)        # AV(7), lag 1
                if head > 0 and c == TCHUNK - 1:
                    emit_fin(head - 1)            # fin 0..6
            # tail
            emit_av(NH - 1, TCHUNK - 1)
            emit_fin(NH - 1, fast_tail=True)

    nc.compile()
    return nc


def _get_nc():
    if "nc" not in _CACHE:
        _CACHE["nc"] = _build()
    return _CACHE["nc"]


def build_in_maps(inputs):
    query, key, value = inputs["query"], inputs["key"], inputs["value"]
    f = np.float32
    B = query.shape[0]

    def pack_w(w):
        # [NJ, 128(p=c_in%128), NKC, 128(m)]: w[j*128+m, kc*128+p]
        wa = np.asarray(w, dtype=f).astype(np.float16)
        wa = wa.reshape(NJ, 128, NKC, 128)        # [j, m, kc, p]
        return np.ascontiguousarray(wa.transpose(0, 3, 2, 1))

    def pack_wv(w):
        wa = np.asarray(w, dtype=f).astype(np.float16)  # [c_out, c_in]
        wa = wa.T.reshape(NKC, 128, C)            # [kc, p, c_out]
        return np.ascontiguousarray(wa.transpose(1, 0, 2))

    def pack_x(x):
        # [C, S] -> [128, NKC, S]
        xa = np.asarray(x, dtype=f).reshape(NKC, 128, S).astype(np.float16)
        return np.ascontiguousarray(xa.transpose(1, 0, 2))

    wq_p = pack_w(inputs["wq"])
    wk_p = pack_w(inputs["wk"])
    wv_p = pack_wv(inputs["wv"])
    bq_p = np.ascontiguousarray(
        np.asarray(inputs["bq"], dtype=f).reshape(NJ, 128).T)
    bvb_p = np.ascontiguousarray(
        np.broadcast_to(np.asarray(inputs["bv"], dtype=f)[None, :], (128, C)))

    in_maps = []
    for b in range(B):
        xq_p = pack_x(np.asarray(query[b], dtype=f).reshape(C, S))
        xk_p = pack_x(np.asarray(key[b], dtype=f).reshape(C, S))
        xv_p = pack_x(np.asarray(value[b], dtype=f).reshape(C, S))
        in_maps.append({
            "xqk": np.ascontiguousarray(
                np.concatenate([xq_p, xk_p], axis=1)),
            "xv": xv_p,
            "wq": wq_p, "wk": wk_p, "wv": wv_p,
            "bqd": bq_p, "bvb": bvb_p,
        })
    return in_maps


def kernel(query, key, value, wq, bq, wk, bk, wv, bv):
    nc = _get_nc()
    B = query.shape[0]
    assert B == N_CORES

    in_maps = build_in_maps({
        "query": query, "key": key, "value": value,
        "wq": wq, "bq": bq, "wk": wk, "bk": bk, "wv": wv, "bv": bv,
    })

    res = bass_utils.run_bass_kernel_spmd(nc, in_maps, core_ids=list(range(B)))
    _CACHE["last_result"] = res
    outs = [res.results[b]["out"].reshape(C, 32, 32) for b in range(B)]
    return np.stack(outs).astype(np.float32)
